# revision 9
# baseline (speedup 1.0000x reference)
"""Radon transform (bilinear grid-sample + row-sum) on 8 TRN2 NeuronCores.

v2 bundle design: 4 adjacent sinogram bins share one tile stream (their
sample lines are ~1-1.4 cols apart in the variant frame, so one d x 8
gathered tile serves all four).  ap_gather fetches each bundle's tiles
once (GPSIMD ~35ns/idx is a main bottleneck, so sharing tiles across 4
bins cuts the index count ~2.5x).  DVE runs four 2x-mode bf16 multiplies
per chunk (one per bin-offset r, weights precomputed on host, in-place
over the weight buffer).  The segment reduction runs on the idle
TensorEngine: one PSUM-accumulating matmul per (chunk, r) whose moving
operand iterates (l, x) and whose output AP broadcasts over l (stride-0),
so each bundle column accumulates L*d products into psum[2, r*96+x] while
sel (128x2) folds the 8 groups * 8 jc * 2 batches partition dim.  Scalar
copies psum to SBUF and DMAs [2, 384] per slot; the host unshuffles
(bundle, r) -> bin.

All gather indices / weights are input-independent and precomputed on host.
"""
import math
import os
import sys
from contextlib import ExitStack

import numpy as np

sys.path.insert(0, "/opt/trn_rl_repo")

import ml_dtypes  # noqa: E402

BF16 = ml_dtypes.bfloat16

# ─── geometry constants (hardcoded for 256x256, 180 angles, batch 2) ───
N_ANGLES = 180
IMG_SIZE = 256
BATCH = 2
S = int(math.ceil(math.sqrt(2.0) * IMG_SIZE))  # 363

ROWS = 384            # slab rows (multiple of 32)
XT = 92               # col-tiles per partition slice
NELS = XT * ROWS      # 35328 elements per partition
NCOL = XT * 8         # 736 layout columns
KB = 2                # bins per bundle
NBUND = 192           # padded bundle count (ceil(363/2)=182 -> 192)
SXPAD = KB * NBUND    # 384 psum/sino columns: col = r*NBUND + bundle
SLOTS = 24            # slot 0 is a dummy pipeline-priming slot
DUMMY_LSEG = 1
NGROUP = 8
D = 16                # gather block depth (rows per tile)
NRB = ROWS // D       # 24 row-blocks
CHUNK_NXG = [48] * 4  # bundles per chunk (48*4 = 192 = NBUND)

# angle classes: (variant, angle list). variant row/col maps:
#   v0: row=Y+1, col=X+1            (|tan| <= tan22.5, theta near 0/180)
#   v1: row=X+1, col=Y+1            (theta near 90)
#   v2a: row=Y+1, col=X-Y+368       (22.5 < th <= 45)
#   v2b: row=X+1, col=X-Y+368       (45 < th < 67.5)
#   v3a: row=X+1, col=X+Y+2         (112.5 < th <= 135)
#   v3b: row=Y+1, col=X+Y+2         (135 < th < 157.5)
CORE_SPECS = [
    ("v0", list(range(0, 23))),
    ("v0", list(range(158, 180))),
    ("v2a", list(range(23, 46))),
    ("v2b", list(range(46, 68))),
    ("v1", list(range(68, 91))),
    ("v1", list(range(91, 113))),
    ("v3a", list(range(113, 136))),
    ("v3b", list(range(136, 158))),
]


def _angle_tables(k):
    theta = np.float32(k) * np.float32(np.pi / N_ANGLES)
    c = np.cos(theta, dtype=np.float32)
    s = np.sin(theta, dtype=np.float32)
    lin = np.linspace(-1.0, 1.0, S, dtype=np.float32)
    x = lin[None, :]
    y = lin[:, None]
    gx = c * x + s * y
    gy = -s * x + c * y
    ix = ((gx + np.float32(1.0)) * np.float32(0.5) * np.float32(S - 1)).astype(np.float32)
    iy = ((gy + np.float32(1.0)) * np.float32(0.5) * np.float32(S - 1)).astype(np.float32)
    x0 = np.floor(ix)
    y0 = np.floor(iy)
    wx = ix - x0
    wy = iy - y0
    return y0.astype(np.int64), x0.astype(np.int64), wx, wy


def _variant_rc(var, Y, X):
    if var == "v0":
        return Y + 1, X + 1
    if var == "v1":
        return X + 1, Y + 1
    if var == "v2a":
        return Y + 1, X - Y + 368
    if var == "v2b":
        return X + 1, X - Y + 368
    if var == "v3a":
        return X + 1, X + Y + 2
    return Y + 1, X + Y + 2  # v3b


def _angle_taps(k, var):
    """Flat arrays over valid taps: bin, samp, row, col, w (f32)."""
    y0, x0, wx, wy = _angle_tables(k)
    kk = np.broadcast_to(np.arange(S, dtype=np.int64)[:, None], (S, S))
    bb = np.broadcast_to(np.arange(S, dtype=np.int64)[None, :], (S, S))
    outs = []
    for dy in (0, 1):
        for dx in (0, 1):
            Y = y0 + dy
            X = x0 + dx
            w = (wy if dy else (1.0 - wy)) * (wx if dx else (1.0 - wx))
            v = (Y >= 0) & (Y < S) & (X >= 0) & (X < S)
            r, c = _variant_rc(var, Y, X)
            outs.append((bb[v], kk[v], r[v], c[v], w[v].astype(np.float32)))
    bins = np.concatenate([o[0] for o in outs])
    samp = np.concatenate([o[1] for o in outs])
    row = np.concatenate([o[2] for o in outs])
    col = np.concatenate([o[3] for o in outs])
    w = np.concatenate([o[4] for o in outs])
    return bins, samp, row, col, w


def _angle_bundles(k, var):
    """Unique (bundle, tile) set + per-tap mapping helpers.

    tile = xt * NRB + rb; bundles ordered; tiles within bundle ordered by
    (rb, xt) (row sweep); dealt round-robin to 8 groups.
    """
    bins, samp, row, col, w = _angle_taps(k, var)
    bund = bins // KB
    r_off = bins % KB
    rb = row // D
    xt = col // 8
    jc = col % 8
    e = row % D
    tile = xt * NRB + rb                      # block index in slab
    # order tiles within bundle by (rb, xt)
    okey = bund * (NRB * XT) + rb * XT + xt
    uok = np.unique(okey)
    ubund = uok // (NRB * XT)
    urb = (uok // XT) % NRB
    uxt = uok % XT
    utile = uxt * NRB + urb
    starts = np.searchsorted(ubund, np.arange(NBUND + 1))
    pos = np.arange(len(ubund)) - starts[ubund]
    ntiles = np.diff(starts)                  # tiles per bundle
    grp_u = pos % NGROUP
    t_u = pos // NGROUP
    # per-chunk lsegs
    edges = np.cumsum([0] + CHUNK_NXG)
    lsegs = []
    for ci in range(len(CHUNK_NXG)):
        lo, hi = edges[ci], edges[ci + 1]
        nt = ntiles[lo:hi]
        lsegs.append(max(int(np.ceil(nt.max() / NGROUP)) if len(nt) else 1, 1))
    # per-tap lookup: unique-key row via searchsorted on okey space
    tap_okey = bund * (NRB * XT) + rb * XT + xt
    urow = np.searchsorted(uok, tap_okey)
    return dict(lsegs=lsegs, uok=uok, ubund=ubund, utile=utile,
                grp_u=grp_u, t_u=t_u, ntiles=ntiles, urow=urow,
                tap_bund=bund, tap_r=r_off, tap_jc=jc, tap_e=e, tap_w=w)


_PLAN_CACHE = {}


def _get_plan():
    if "plan" in _PLAN_CACHE:
        return _PLAN_CACHE["plan"]
    # pass A: per (core, angle): lsegs + cost -> per-core angle order,
    # global lseg_table (cross-core max per slot).
    info = {}
    for ci, (var, angles) in enumerate(CORE_SPECS):
        for k in angles:
            a = _angle_bundles(k, var)
            # lsegs from DESC-sorted ntiles (bundles packed by rank)
            snt = np.sort(a["ntiles"])[::-1]
            edges0 = np.cumsum([0] + CHUNK_NXG)
            ls = []
            for cix in range(len(CHUNK_NXG)):
                mx = snt[edges0[cix]:edges0[cix + 1]].max()
                ls.append(max(int(np.ceil(mx / NGROUP)), 1))
            info[k] = ls
    core_order = []
    for ci, (var, angles) in enumerate(CORE_SPECS):
        cost = {k: sum(info[k]) for k in angles}
        core_order.append(sorted(angles, key=lambda k: -cost[k]))

    NSLOT_A = SLOTS - 1
    lseg_table = [[DUMMY_LSEG] * len(CHUNK_NXG)]
    for si in range(NSLOT_A):
        ls = [1] * len(CHUNK_NXG)
        for ci in range(8):
            if si < len(core_order[ci]):
                al = info[core_order[ci][si]]
                ls = [max(a, b) for a, b in zip(ls, al)]
        lseg_table.append(ls)

    # stream layout: per slot, per chunk: cn = nxg * L indices, cnd els
    chunks = []
    o16 = ow = 0
    for si in range(SLOTS):
        for cidx, nxg in enumerate(CHUNK_NXG):
            L = lseg_table[si][cidx]
            cn = nxg * L
            chunks.append(dict(si=si, cidx=cidx, L=L, cn=cn, nxg=nxg,
                               xoff=sum(CHUNK_NXG[:cidx]), o16=o16, ow=ow))
            o16 += cn // 16
            ow += cn * D
    tot16, totw = o16, ow
    maxcn = max(ch["cn"] for ch in chunks)
    maxels = maxcn * D

    # pass B: per-core idx blobs [128, tot16] + w blobs [64, 4*totw]
    core_idx = []
    core_w = []
    core_perm = []
    for ci, (var, angles) in enumerate(CORE_SPECS):
        idx_blob = np.zeros((128, tot16), np.int16)
        w_blob = np.zeros((64, KB * totw), np.float32)
        perms = {}
        for si in range(SLOTS):
            if si == 0 or si - 1 >= len(core_order[ci]):
                continue
            k = core_order[ci][si - 1]
            a = _angle_bundles(k, var)
            lsegs = lseg_table[si]
            sch = [c for c in chunks if c["si"] == si]
            edges = np.cumsum([0] + CHUNK_NXG)
            # rank bundles by ntiles desc; bundle -> rank position
            perm = np.argsort(-a["ntiles"], kind="stable")  # rank -> bundle
            rank = np.empty(NBUND, np.int64)
            rank[perm] = np.arange(NBUND)
            perms[si] = perm
            ub0, ut = a["ubund"], a["utile"]
            ub = rank[ub0]
            grp, tt = a["grp_u"], a["t_u"]
            cid = np.searchsorted(edges, ub, side="right") - 1
            L_arr = np.array([lsegs[c] for c in range(len(CHUNK_NXG))])
            off_arr = np.array([sch[c]["o16"] * 16 for c in range(len(CHUNK_NXG))])
            assert np.all(tt < L_arr[cid]), (ci, si, k, tt.max())
            spos = off_arr[cid] + (ub - edges[cid]) * L_arr[cid] + tt
            # scatter idx values: stream for group g wrapped into
            # partitions 16g..16g+15: idx[16g + (p%16), p//16] = val
            sv = np.zeros((NGROUP, tot16 * 16), np.int16)
            filled = np.zeros((NGROUP, tot16 * 16), bool)
            sv[grp, spos] = ut
            filled[grp, spos] = True
            lo16 = sch[0]["o16"] * 16
            hi16 = (sch[-1]["o16"] + sch[-1]["cn"] // 16) * 16
            for g in range(NGROUP):
                seg = sv[g, lo16:hi16]
                fil = filled[g, lo16:hi16]
                idxs = np.where(fil, np.arange(len(seg)), 0)
                np.maximum.accumulate(idxs, out=idxs)
                sv[g, lo16:hi16] = seg[idxs]
            for g in range(NGROUP):
                st = sv[g, lo16:hi16]
                wrap = st.reshape(-1, 16).T
                idx_blob[16 * g:16 * g + 16, lo16 // 16:hi16 // 16] = wrap
            # weights: per tap: chunk block [64, KB*cnd] at 4*ow;
            # inside: r*cnd + q*D + e  (q = stream pos within chunk)
            urow = a["urow"]
            tap_grp = grp[urow]
            tap_t = tt[urow]
            tap_bund = rank[a["tap_bund"]]
            tap_cid = np.searchsorted(edges, tap_bund, side="right") - 1
            tap_spos = (off_arr[tap_cid] + (tap_bund - edges[tap_cid])
                        * L_arr[tap_cid] + tap_t)
            o16_arr = np.array([sch[c]["o16"] * 16 for c in range(len(CHUNK_NXG))])
            ow_arr = np.array([sch[c]["ow"] for c in range(len(CHUNK_NXG))])
            cnd_arr = np.array([sch[c]["cn"] * D for c in range(len(CHUNK_NXG))])
            q = tap_spos - o16_arr[tap_cid]
            wrow = 8 * tap_grp + a["tap_jc"]
            wcol = (KB * ow_arr[tap_cid] + a["tap_r"] * cnd_arr[tap_cid]
                    + q * D + a["tap_e"])
            np.add.at(w_blob, (wrow, wcol), a["tap_w"])
        core_idx.append(idx_blob)
        core_w.append(w_blob.astype(BF16))
        core_perm.append(perms)

    sel = np.zeros((128, 2), np.float32)
    for p in range(128):
        sel[p, p % 2] = 1.0
    plan = dict(lseg_table=lseg_table, chunks=chunks, tot16=tot16,
                totw=totw, maxcn=maxcn, maxels=maxels, core_idx=core_idx,
                core_w=core_w, sel=sel, core_order=core_order,
                core_perm=core_perm)
    _PLAN_CACHE["plan"] = plan
    return plan


def _build_slab(image, var):
    """[128, NELS] bf16: partition p=(g,jc,b): cols ≡ jc mod 8 of variant
    frame, batch b; element idx = xt*ROWS + row."""
    img = np.asarray(image, np.float32)[:, 0]
    pad_total = S - IMG_SIZE
    pb = pad_total // 2
    pimg = np.zeros((BATCH, S, S), np.float32)
    pimg[:, pb:pb + IMG_SIZE, pb:pb + IMG_SIZE] = img
    Yg, Xg = np.meshgrid(np.arange(S), np.arange(S), indexing="ij")
    r, c = _variant_rc(var, Yg, Xg)
    frame = np.zeros((BATCH, ROWS, NCOL), np.float32)
    frame[:, r, c] = pimg
    slab16 = np.zeros((16, NELS), np.float32)
    for jc in range(8):
        cols = frame[:, :, jc::8]              # [B, ROWS, XT]
        sl = np.transpose(cols, (0, 2, 1)).reshape(BATCH, -1)  # xt-major rows
        for b in range(BATCH):
            slab16[2 * jc + b] = sl[b]
    return np.tile(slab16, (8, 1)).astype(BF16)


_PROG_CACHE = {}


def _build_program(plan):
    if "prog" in _PROG_CACHE:
        return _PROG_CACHE["prog"]
    import concourse.bass as bass
    import concourse.mybir as mybir
    from concourse import library_config

    chunks = plan["chunks"]
    maxcn = plan["maxcn"]
    maxels = plan["maxels"]

    nc = bass.Bass()
    slab_d = nc.declare_dram_parameter("slab", [128, NELS],
                                       mybir.dt.bfloat16, isOutput=False)
    idx_d = nc.declare_dram_parameter("idx", [128, plan["tot16"]],
                                      mybir.dt.int16, isOutput=False)
    w_d = nc.declare_dram_parameter("w", [64, KB * plan["totw"]],
                                    mybir.dt.bfloat16, isOutput=False)
    sel_d = nc.declare_dram_parameter("sel", [128, 2], mybir.dt.float32,
                                      isOutput=False)
    out_d = nc.declare_dram_parameter("out", [SLOTS, 2, SXPAD],
                                      mybir.dt.float32, isOutput=True)

    ctx = ExitStack()
    with ctx:
        slab_t = ctx.enter_context(nc.sbuf_tensor([128, NELS], mybir.dt.bfloat16))
        idx_ts = [ctx.enter_context(nc.sbuf_tensor(f"idx{i}", [128, maxcn // 16], mybir.dt.int16)) for i in range(4)]
        w_ts = [ctx.enter_context(nc.sbuf_tensor(f"w{i}", [128, KB * maxels], mybir.dt.bfloat16)) for i in range(3)]
        g_ts = [ctx.enter_context(nc.sbuf_tensor(f"g{i}", [128, maxels], mybir.dt.bfloat16)) for i in range(3)]
        sel_t = ctx.enter_context(nc.sbuf_tensor([128, 2], mybir.dt.float32))
        r_ts = [ctx.enter_context(nc.sbuf_tensor(f"r{i}", [128, SXPAD], mybir.dt.float32)) for i in range(2)]
        sino_ts = [ctx.enter_context(nc.sbuf_tensor(f"sino{i}", [2, SXPAD], mybir.dt.float32)) for i in range(2)]
        psum_ts = [ctx.enter_context(nc.psum_tensor(f"ps{i}", [2, SXPAD], mybir.dt.float32)) for i in range(2)]
        s_in = ctx.enter_context(nc.semaphore("s_in"))
        s_di = ctx.enter_context(nc.semaphore("s_di"))
        s_dw = ctx.enter_context(nc.semaphore("s_dw"))
        s_g = ctx.enter_context(nc.semaphore("s_g"))
        s_v = ctx.enter_context(nc.semaphore("s_v"))
        s_mm = ctx.enter_context(nc.semaphore("s_mm"))
        s_cp = ctx.enter_context(nc.semaphore("s_cp"))
        s_od = ctx.enter_context(nc.semaphore("s_od"))
        block = ctx.enter_context(nc.Block())

        slot_end = [0] * SLOTS
        for n, ch in enumerate(chunks):
            slot_end[ch["si"]] = n + 1

        @block.sync
        def _(sync):
            sync.dma_start(out=sel_t[:], in_=sel_d[:]).then_inc(s_in, 16)
            sync.dma_start(out=slab_t[:], in_=slab_d[:]).then_inc(s_in, 16)
            for n, ch in enumerate(chunks):
                cnd = ch["cn"] * D
                # idx buf n%4 read by gather n; w buf n%3 read by vector
                # mult n and tensor matmuls n (freed at s_mm >= n+1)
                if n > 3:
                    sync.wait_ge(s_g, n - 3)
                if n > 2:
                    sync.wait_ge(s_v, n - 2)
                sync.dma_start(
                    out=idx_ts[n % 4][:, :ch["cn"] // 16],
                    in_=idx_d[:, ch["o16"]:ch["o16"] + ch["cn"] // 16],
                ).then_inc(s_di, 16)
                wsrc = (w_d[:, KB * ch["ow"]:KB * ch["ow"] + KB * cnd]
                        .unsqueeze(1).broadcast_to([64, 2, KB * cnd]))
                sync.dma_start(out=w_ts[n % 3][:, :KB * cnd], in_=wsrc
                               ).then_inc(s_dw, 16)

        @block.gpsimd
        def _(g):
            g.load_library(library_config.ap_gather)
            g.wait_ge(s_in, 32)
            g.wait_ge(s_di, 16)
            # warmup: amortize ext-isa first-call cost + preamble margin
            ch0 = chunks[0]
            g.ap_gather(
                g_ts[2][:, :ch0["cn"] * D].rearrange("p (n d) -> p n d", d=D),
                slab_t[:].rearrange("p (n d) -> p n d", d=D),
                idx_ts[0][:, :ch0["cn"] // 16],
                channels=128, num_elems=NELS // D, d=D, num_idxs=ch0["cn"],
            )
            for n, ch in enumerate(chunks):
                g.wait_ge(s_di, 16 * (n + 1))
                if n > 2:
                    g.wait_ge(s_v, n - 2)  # g_ts[n%3] consumed by vector n
                g.ap_gather(
                    g_ts[n % 3][:, :ch["cn"] * D].rearrange("p (n d) -> p n d", d=D),
                    slab_t[:].rearrange("p (n d) -> p n d", d=D),
                    idx_ts[n % 4][:, :ch["cn"] // 16],
                    channels=128, num_elems=NELS // D, d=D, num_idxs=ch["cn"],
                ).then_inc(s_g, 1)

        @block.vector
        def _(v):
            for n, ch in enumerate(chunks):
                cnd = ch["cn"] * D
                si = ch["si"]
                LD = ch["L"] * D
                v.wait_ge(s_g, n + 1)
                v.wait_ge(s_dw, 16 * (n + 1))
                if ch["cidx"] == 0 and si > 1:
                    v.wait_ge(s_mm, si - 1)  # r_ts freed by slot matmul
                rdst = r_ts[si % 2]
                wv = w_ts[n % 3][:, :KB * cnd].rearrange(
                    "p (r c) -> p r c", r=KB)
                gbc = (g_ts[n % 3][:, :cnd].unsqueeze(1)
                       .broadcast_to([128, KB, cnd]))
                v.tensor_mul(wv, gbc, wv)
                v.tensor_reduce(
                    out=rdst[:].rearrange("p (r b) -> p r b", r=KB)[
                        :, :, ch["xoff"]:ch["xoff"] + ch["nxg"]],
                    in_=w_ts[n % 3][:, :KB * cnd].rearrange(
                        "p (r x l) -> p r x l", r=KB, l=LD),
                    axis=mybir.AxisListType.X,
                    op=mybir.AluOpType.add,
                ).then_inc(s_v, 1)

        @block.tensor
        def _(t):
            t.wait_ge(s_in, 32)
            for si in range(SLOTS):
                t.wait_ge(s_v, slot_end[si])
                if si > 1:
                    t.wait_ge(s_cp, si - 1)  # psum freed by scalar copy
                t.matmul(psum_ts[si % 2][:], sel_t[:], r_ts[si % 2][:],
                         start=True, stop=True).then_inc(s_mm, 1)

        @block.scalar
        def _(sc):
            for si in range(SLOTS):
                sc.wait_ge(s_mm, si + 1)
                if si > 1:
                    sc.wait_ge(s_od, 16 * (si - 1))  # sino buf freed by DMA
                sc.copy(sino_ts[si % 2][:], psum_ts[si % 2][:]).then_inc(s_cp, 1)
                sc.dma_start(out=out_d[si], in_=sino_ts[si % 2][:]
                             ).then_inc(s_od, 16)
            sc.wait_ge(s_od, 16 * SLOTS)

    import concourse.mybir as mybir2
    mybir2.codegen_inst_isa_subclasses(nc)
    _PROG_CACHE["prog"] = nc
    return nc


def kernel(image):
    image = np.asarray(image, np.float32)
    assert image.shape == (BATCH, 1, IMG_SIZE, IMG_SIZE)
    plan = _get_plan()
    nc = _build_program(plan)

    from concourse.bass_utils import run_bass_kernel_spmd

    in_maps = []
    for ci, (var, angles) in enumerate(CORE_SPECS):
        in_maps.append({
            "slab": _build_slab(image, var),
            "idx": plan["core_idx"][ci],
            "w": plan["core_w"][ci],
            "sel": plan["sel"],
        })

    trace = bool(os.environ.get("RADON_TRACE"))
    if trace:
        _install_profhook()
    res = run_bass_kernel_spmd(nc, in_maps, list(range(8)), trace=trace)
    if trace:
        kernel.last_exec_time_ns = res.exec_time_ns

    sino = np.zeros((BATCH, 1, S, N_ANGLES), np.float32)
    for ci in range(8):
        o = res.results[ci]["out"]  # [SLOTS, 2, SXPAD]; slot 0 is dummy
        for si, k in enumerate(plan["core_order"][ci]):
            v = o[si + 1]  # [2, SXPAD]: col = r*NBUND + rankpos
            perm = plan["core_perm"][ci][si + 1]    # rankpos -> bundle
            full = v.reshape(2, KB, NBUND)          # [2, r, rankpos]
            binidx = perm[None, :] * KB + np.arange(KB)[:, None]  # [KB, rank]
            mask = binidx < S
            sino[:, 0, binidx[mask], k] = full[:, mask]
    return sino


def _install_profhook():
    import types
    if "antenv.axon_hooks" in sys.modules:
        return
    try:
        from trn_agent_boot.trn_boot import _ntff_profile_via_ctypes
        hook = _ntff_profile_via_ctypes("/opt/axon/libaxon_pjrt.so")
    except Exception:
        hook = None
    mod = types.ModuleType("antenv.axon_hooks")
    mod._hook = hook
    mod.set_axon_ntff_profile_hook = lambda h: setattr(mod, "_hook", h)
    mod.get_axon_ntff_profile_hook = lambda: mod._hook
    sys.modules["antenv.axon_hooks"] = mod
    import antenv
    antenv.axon_hooks = mod


if __name__ == "__main__":
    img = np.load("/tmp/ref_image.npy")
    out = kernel(image=img)
    exp = np.load("/tmp/ref_expected.npy")
    err = np.linalg.norm(out - exp) / np.linalg.norm(exp)
    print("kernel rel err:", err)


# revision 12
# speedup vs baseline: 1.0029x; 1.0029x over previous
"""Radon transform (bilinear grid-sample + row-sum) on 8 TRN2 NeuronCores.

v2 bundle design: 4 adjacent sinogram bins share one tile stream (their
sample lines are ~1-1.4 cols apart in the variant frame, so one d x 8
gathered tile serves all four).  ap_gather fetches each bundle's tiles
once (GPSIMD ~35ns/idx is a main bottleneck, so sharing tiles across 4
bins cuts the index count ~2.5x).  DVE runs four 2x-mode bf16 multiplies
per chunk (one per bin-offset r, weights precomputed on host, in-place
over the weight buffer).  The segment reduction runs on the idle
TensorEngine: one PSUM-accumulating matmul per (chunk, r) whose moving
operand iterates (l, x) and whose output AP broadcasts over l (stride-0),
so each bundle column accumulates L*d products into psum[2, r*96+x] while
sel (128x2) folds the 8 groups * 8 jc * 2 batches partition dim.  Scalar
copies psum to SBUF and DMAs [2, 384] per slot; the host unshuffles
(bundle, r) -> bin.

All gather indices / weights are input-independent and precomputed on host.
"""
import math
import os
import sys
from contextlib import ExitStack

import numpy as np

sys.path.insert(0, "/opt/trn_rl_repo")

import ml_dtypes  # noqa: E402

BF16 = ml_dtypes.bfloat16

# ─── geometry constants (hardcoded for 256x256, 180 angles, batch 2) ───
N_ANGLES = 180
IMG_SIZE = 256
BATCH = 2
S = int(math.ceil(math.sqrt(2.0) * IMG_SIZE))  # 363

ROWS = 384            # slab rows (multiple of 32)
XT = 92               # col-tiles per partition slice
NELS = XT * ROWS      # 35328 elements per partition
NCOL = XT * 8         # 736 layout columns
KB = 2                # bins per bundle
NBUND = 192           # padded bundle count (ceil(363/2)=182 -> 192)
SXPAD = KB * NBUND    # 384 psum/sino columns: col = r*NBUND + bundle
SLOTS = 24            # slot 0 is a dummy pipeline-priming slot
DUMMY_LSEG = 1
NGROUP = 8
D = 16                # gather block depth (rows per tile)
NRB = ROWS // D       # 24 row-blocks
CHUNK_NXG = [48] * 4  # bundles per chunk (48*4 = 192 = NBUND)

# angle classes: (variant, angle list). variant row/col maps:
#   v0: row=Y+1, col=X+1            (|tan| <= tan22.5, theta near 0/180)
#   v1: row=X+1, col=Y+1            (theta near 90)
#   v2a: row=Y+1, col=X-Y+368       (22.5 < th <= 45)
#   v2b: row=X+1, col=X-Y+368       (45 < th < 67.5)
#   v3a: row=X+1, col=X+Y+2         (112.5 < th <= 135)
#   v3b: row=Y+1, col=X+Y+2         (135 < th < 157.5)
CORE_SPECS = [
    ("v0", list(range(0, 23))),
    ("v0", list(range(158, 180))),
    ("v2a", list(range(23, 46))),
    ("v2b", list(range(46, 68))),
    ("v1", list(range(68, 91))),
    ("v1", list(range(91, 113))),
    ("v3a", list(range(113, 136))),
    ("v3b", list(range(136, 158))),
]


def _angle_tables(k):
    theta = np.float32(k) * np.float32(np.pi / N_ANGLES)
    c = np.cos(theta, dtype=np.float32)
    s = np.sin(theta, dtype=np.float32)
    lin = np.linspace(-1.0, 1.0, S, dtype=np.float32)
    x = lin[None, :]
    y = lin[:, None]
    gx = c * x + s * y
    gy = -s * x + c * y
    ix = ((gx + np.float32(1.0)) * np.float32(0.5) * np.float32(S - 1)).astype(np.float32)
    iy = ((gy + np.float32(1.0)) * np.float32(0.5) * np.float32(S - 1)).astype(np.float32)
    x0 = np.floor(ix)
    y0 = np.floor(iy)
    wx = ix - x0
    wy = iy - y0
    return y0.astype(np.int64), x0.astype(np.int64), wx, wy


def _variant_rc(var, Y, X):
    if var == "v0":
        return Y + 1, X + 1
    if var == "v1":
        return X + 1, Y + 1
    if var == "v2a":
        return Y + 1, X - Y + 368
    if var == "v2b":
        return X + 1, X - Y + 368
    if var == "v3a":
        return X + 1, X + Y + 2
    return Y + 1, X + Y + 2  # v3b


def _angle_taps(k, var):
    """Flat arrays over valid taps: bin, samp, row, col, w (f32)."""
    y0, x0, wx, wy = _angle_tables(k)
    kk = np.broadcast_to(np.arange(S, dtype=np.int64)[:, None], (S, S))
    bb = np.broadcast_to(np.arange(S, dtype=np.int64)[None, :], (S, S))
    outs = []
    for dy in (0, 1):
        for dx in (0, 1):
            Y = y0 + dy
            X = x0 + dx
            w = (wy if dy else (1.0 - wy)) * (wx if dx else (1.0 - wx))
            v = (Y >= 0) & (Y < S) & (X >= 0) & (X < S)
            r, c = _variant_rc(var, Y, X)
            outs.append((bb[v], kk[v], r[v], c[v], w[v].astype(np.float32)))
    bins = np.concatenate([o[0] for o in outs])
    samp = np.concatenate([o[1] for o in outs])
    row = np.concatenate([o[2] for o in outs])
    col = np.concatenate([o[3] for o in outs])
    w = np.concatenate([o[4] for o in outs])
    return bins, samp, row, col, w


def _angle_bundles(k, var):
    """Unique (bundle, tile) set + per-tap mapping helpers.

    tile = xt * NRB + rb; bundles ordered; tiles within bundle ordered by
    (rb, xt) (row sweep); dealt round-robin to 8 groups.
    """
    bins, samp, row, col, w = _angle_taps(k, var)
    bund = bins // KB
    r_off = bins % KB
    rb = row // D
    xt = col // 8
    jc = col % 8
    e = row % D
    tile = xt * NRB + rb                      # block index in slab
    # order tiles within bundle by (rb, xt)
    okey = bund * (NRB * XT) + rb * XT + xt
    uok = np.unique(okey)
    ubund = uok // (NRB * XT)
    urb = (uok // XT) % NRB
    uxt = uok % XT
    utile = uxt * NRB + urb
    starts = np.searchsorted(ubund, np.arange(NBUND + 1))
    pos = np.arange(len(ubund)) - starts[ubund]
    ntiles = np.diff(starts)                  # tiles per bundle
    grp_u = pos % NGROUP
    t_u = pos // NGROUP
    # per-chunk lsegs
    edges = np.cumsum([0] + CHUNK_NXG)
    lsegs = []
    for ci in range(len(CHUNK_NXG)):
        lo, hi = edges[ci], edges[ci + 1]
        nt = ntiles[lo:hi]
        lsegs.append(max(int(np.ceil(nt.max() / NGROUP)) if len(nt) else 1, 1))
    # per-tap lookup: unique-key row via searchsorted on okey space
    tap_okey = bund * (NRB * XT) + rb * XT + xt
    urow = np.searchsorted(uok, tap_okey)
    return dict(lsegs=lsegs, uok=uok, ubund=ubund, utile=utile,
                grp_u=grp_u, t_u=t_u, ntiles=ntiles, urow=urow,
                tap_bund=bund, tap_r=r_off, tap_jc=jc, tap_e=e, tap_w=w)


_PLAN_CACHE = {}


def _get_plan():
    if "plan" in _PLAN_CACHE:
        return _PLAN_CACHE["plan"]
    # pass A: per (core, angle): lsegs + cost -> per-core angle order,
    # global lseg_table (cross-core max per slot).
    info = {}
    for ci, (var, angles) in enumerate(CORE_SPECS):
        for k in angles:
            a = _angle_bundles(k, var)
            # lsegs from DESC-sorted ntiles (bundles packed by rank)
            snt = np.sort(a["ntiles"])[::-1]
            edges0 = np.cumsum([0] + CHUNK_NXG)
            ls = []
            for cix in range(len(CHUNK_NXG)):
                mx = snt[edges0[cix]:edges0[cix + 1]].max()
                ls.append(max(int(np.ceil(mx / NGROUP)), 1))
            info[k] = ls
    core_order = []
    for ci, (var, angles) in enumerate(CORE_SPECS):
        cost = {k: sum(info[k]) for k in angles}
        core_order.append(sorted(angles, key=lambda k: -cost[k]))

    NSLOT_A = SLOTS - 1
    lseg_table = [[DUMMY_LSEG] * len(CHUNK_NXG)]
    for si in range(NSLOT_A):
        ls = [1] * len(CHUNK_NXG)
        for ci in range(8):
            if si < len(core_order[ci]):
                al = info[core_order[ci][si]]
                ls = [max(a, b) for a, b in zip(ls, al)]
        lseg_table.append(ls)

    # stream layout: per slot, per chunk: cn = nxg * L indices, cnd els
    chunks = []
    o16 = ow = 0
    for si in range(SLOTS):
        for cidx, nxg in enumerate(CHUNK_NXG):
            L = lseg_table[si][cidx]
            cn = nxg * L
            chunks.append(dict(si=si, cidx=cidx, L=L, cn=cn, nxg=nxg,
                               xoff=sum(CHUNK_NXG[:cidx]), o16=o16, ow=ow))
            o16 += cn // 16
            ow += cn * D
    tot16, totw = o16, ow
    maxcn = max(ch["cn"] for ch in chunks)
    maxels = maxcn * D

    # pass B: per-core idx blobs [128, tot16] + w blobs [64, 4*totw]
    core_idx = []
    core_w = []
    core_perm = []
    for ci, (var, angles) in enumerate(CORE_SPECS):
        idx_blob = np.zeros((128, tot16), np.int16)
        w_blob = np.zeros((64, KB * totw), np.float32)
        perms = {}
        for si in range(SLOTS):
            if si == 0 or si - 1 >= len(core_order[ci]):
                continue
            k = core_order[ci][si - 1]
            a = _angle_bundles(k, var)
            lsegs = lseg_table[si]
            sch = [c for c in chunks if c["si"] == si]
            edges = np.cumsum([0] + CHUNK_NXG)
            # rank bundles by ntiles desc; bundle -> rank position
            perm = np.argsort(-a["ntiles"], kind="stable")  # rank -> bundle
            rank = np.empty(NBUND, np.int64)
            rank[perm] = np.arange(NBUND)
            perms[si] = perm
            ub0, ut = a["ubund"], a["utile"]
            ub = rank[ub0]
            grp, tt = a["grp_u"], a["t_u"]
            cid = np.searchsorted(edges, ub, side="right") - 1
            L_arr = np.array([lsegs[c] for c in range(len(CHUNK_NXG))])
            off_arr = np.array([sch[c]["o16"] * 16 for c in range(len(CHUNK_NXG))])
            assert np.all(tt < L_arr[cid]), (ci, si, k, tt.max())
            spos = off_arr[cid] + (ub - edges[cid]) * L_arr[cid] + tt
            # scatter idx values: stream for group g wrapped into
            # partitions 16g..16g+15: idx[16g + (p%16), p//16] = val
            sv = np.zeros((NGROUP, tot16 * 16), np.int16)
            filled = np.zeros((NGROUP, tot16 * 16), bool)
            sv[grp, spos] = ut
            filled[grp, spos] = True
            lo16 = sch[0]["o16"] * 16
            hi16 = (sch[-1]["o16"] + sch[-1]["cn"] // 16) * 16
            for g in range(NGROUP):
                seg = sv[g, lo16:hi16]
                fil = filled[g, lo16:hi16]
                idxs = np.where(fil, np.arange(len(seg)), 0)
                np.maximum.accumulate(idxs, out=idxs)
                sv[g, lo16:hi16] = seg[idxs]
            for g in range(NGROUP):
                st = sv[g, lo16:hi16]
                wrap = st.reshape(-1, 16).T
                idx_blob[16 * g:16 * g + 16, lo16 // 16:hi16 // 16] = wrap
            # weights: per tap: chunk block [64, KB*cnd] at 4*ow;
            # inside: r*cnd + q*D + e  (q = stream pos within chunk)
            urow = a["urow"]
            tap_grp = grp[urow]
            tap_t = tt[urow]
            tap_bund = rank[a["tap_bund"]]
            tap_cid = np.searchsorted(edges, tap_bund, side="right") - 1
            tap_spos = (off_arr[tap_cid] + (tap_bund - edges[tap_cid])
                        * L_arr[tap_cid] + tap_t)
            o16_arr = np.array([sch[c]["o16"] * 16 for c in range(len(CHUNK_NXG))])
            ow_arr = np.array([sch[c]["ow"] for c in range(len(CHUNK_NXG))])
            cnd_arr = np.array([sch[c]["cn"] * D for c in range(len(CHUNK_NXG))])
            q = tap_spos - o16_arr[tap_cid]
            wrow = 8 * tap_grp + a["tap_jc"]
            wcol = (KB * ow_arr[tap_cid] + a["tap_r"] * cnd_arr[tap_cid]
                    + q * D + a["tap_e"])
            np.add.at(w_blob, (wrow, wcol), a["tap_w"])
        core_idx.append(idx_blob)
        core_w.append(w_blob.astype(BF16))
        core_perm.append(perms)

    sel = np.zeros((128, 2), np.float32)
    for p in range(128):
        sel[p, p % 2] = 1.0
    plan = dict(lseg_table=lseg_table, chunks=chunks, tot16=tot16,
                totw=totw, maxcn=maxcn, maxels=maxels, core_idx=core_idx,
                core_w=core_w, sel=sel, core_order=core_order,
                core_perm=core_perm)
    _PLAN_CACHE["plan"] = plan
    return plan


def _build_slab(image, var):
    """[128, NELS] bf16: partition p=(g,jc,b): cols ≡ jc mod 8 of variant
    frame, batch b; element idx = xt*ROWS + row."""
    img = np.asarray(image, np.float32)[:, 0]
    pad_total = S - IMG_SIZE
    pb = pad_total // 2
    pimg = np.zeros((BATCH, S, S), np.float32)
    pimg[:, pb:pb + IMG_SIZE, pb:pb + IMG_SIZE] = img
    Yg, Xg = np.meshgrid(np.arange(S), np.arange(S), indexing="ij")
    r, c = _variant_rc(var, Yg, Xg)
    frame = np.zeros((BATCH, ROWS, NCOL), np.float32)
    frame[:, r, c] = pimg
    slab16 = np.zeros((16, NELS), np.float32)
    for jc in range(8):
        cols = frame[:, :, jc::8]              # [B, ROWS, XT]
        sl = np.transpose(cols, (0, 2, 1)).reshape(BATCH, -1)  # xt-major rows
        for b in range(BATCH):
            slab16[2 * jc + b] = sl[b]
    return np.tile(slab16, (8, 1)).astype(BF16)


_PROG_CACHE = {}


def _build_program(plan):
    if "prog" in _PROG_CACHE:
        return _PROG_CACHE["prog"]
    import concourse.bass as bass
    import concourse.mybir as mybir
    from concourse import library_config

    chunks = plan["chunks"]
    maxcn = plan["maxcn"]
    maxels = plan["maxels"]

    nc = bass.Bass()
    slab_d = nc.declare_dram_parameter("slab", [128, NELS],
                                       mybir.dt.bfloat16, isOutput=False)
    idx_d = nc.declare_dram_parameter("idx", [128, plan["tot16"]],
                                      mybir.dt.int16, isOutput=False)
    w_d = nc.declare_dram_parameter("w", [64, KB * plan["totw"]],
                                    mybir.dt.bfloat16, isOutput=False)
    sel_d = nc.declare_dram_parameter("sel", [128, 2], mybir.dt.float32,
                                      isOutput=False)
    out_d = nc.declare_dram_parameter("out", [SLOTS, 2, SXPAD],
                                      mybir.dt.float32, isOutput=True)

    ctx = ExitStack()
    with ctx:
        slab_t = ctx.enter_context(nc.sbuf_tensor([128, NELS], mybir.dt.bfloat16))
        idx_ts = [ctx.enter_context(nc.sbuf_tensor(f"idx{i}", [128, maxcn // 16], mybir.dt.int16)) for i in range(4)]
        w_ts = [ctx.enter_context(nc.sbuf_tensor(f"w{i}", [128, KB * maxels], mybir.dt.bfloat16)) for i in range(3)]
        g_ts = [ctx.enter_context(nc.sbuf_tensor(f"g{i}", [128, maxels], mybir.dt.bfloat16)) for i in range(3)]
        sel_t = ctx.enter_context(nc.sbuf_tensor([128, 2], mybir.dt.float32))
        r_ts = [ctx.enter_context(nc.sbuf_tensor(f"r{i}", [128, SXPAD], mybir.dt.float32)) for i in range(2)]
        sino_ts = [ctx.enter_context(nc.sbuf_tensor(f"sino{i}", [2, SXPAD], mybir.dt.float32)) for i in range(2)]
        psum_ts = [ctx.enter_context(nc.psum_tensor(f"ps{i}", [2, SXPAD], mybir.dt.float32)) for i in range(2)]
        s_in = ctx.enter_context(nc.semaphore("s_in"))
        s_di = ctx.enter_context(nc.semaphore("s_di"))
        s_dw = ctx.enter_context(nc.semaphore("s_dw"))
        s_g = ctx.enter_context(nc.semaphore("s_g"))
        s_v = ctx.enter_context(nc.semaphore("s_v"))
        s_mm = ctx.enter_context(nc.semaphore("s_mm"))
        s_cp = ctx.enter_context(nc.semaphore("s_cp"))
        s_od = ctx.enter_context(nc.semaphore("s_od"))
        block = ctx.enter_context(nc.Block())

        slot_end = [0] * SLOTS
        for n, ch in enumerate(chunks):
            slot_end[ch["si"]] = n + 1

        @block.sync
        def _(sync):
            sync.dma_start(out=sel_t[:], in_=sel_d[:]).then_inc(s_in, 16)
            for n, ch in enumerate(chunks):
                cnd = ch["cn"] * D
                # idx buf n%4 read by gather n; w buf n%3 read by vector
                # mult n and tensor matmuls n (freed at s_mm >= n+1)
                if n > 3:
                    sync.wait_ge(s_g, n - 3)
                if n > 2:
                    sync.wait_ge(s_v, n - 2)
                sync.dma_start(
                    out=idx_ts[n % 4][:, :ch["cn"] // 16],
                    in_=idx_d[:, ch["o16"]:ch["o16"] + ch["cn"] // 16],
                ).then_inc(s_di, 16)
                wsrc = (w_d[:, KB * ch["ow"]:KB * ch["ow"] + KB * cnd]
                        .unsqueeze(1).broadcast_to([64, 2, KB * cnd]))
                sync.dma_start(out=w_ts[n % 3][:, :KB * cnd], in_=wsrc
                               ).then_inc(s_dw, 16)

        @block.gpsimd
        def _(g):
            g.load_library(library_config.ap_gather)
            g.wait_ge(s_in, 32)
            g.wait_ge(s_di, 16)
            # warmup: amortize ext-isa first-call cost + preamble margin
            ch0 = chunks[0]
            g.ap_gather(
                g_ts[2][:, :ch0["cn"] * D].rearrange("p (n d) -> p n d", d=D),
                slab_t[:].rearrange("p (n d) -> p n d", d=D),
                idx_ts[0][:, :ch0["cn"] // 16],
                channels=128, num_elems=NELS // D, d=D, num_idxs=ch0["cn"],
            )
            for n, ch in enumerate(chunks):
                g.wait_ge(s_di, 16 * (n + 1))
                if n > 2:
                    g.wait_ge(s_v, n - 2)  # g_ts[n%3] consumed by vector n
                g.ap_gather(
                    g_ts[n % 3][:, :ch["cn"] * D].rearrange("p (n d) -> p n d", d=D),
                    slab_t[:].rearrange("p (n d) -> p n d", d=D),
                    idx_ts[n % 4][:, :ch["cn"] // 16],
                    channels=128, num_elems=NELS // D, d=D, num_idxs=ch["cn"],
                ).then_inc(s_g, 1)

        @block.vector
        def _(v):
            for n, ch in enumerate(chunks):
                cnd = ch["cn"] * D
                si = ch["si"]
                LD = ch["L"] * D
                v.wait_ge(s_g, n + 1)
                v.wait_ge(s_dw, 16 * (n + 1))
                if ch["cidx"] == 0 and si > 1:
                    v.wait_ge(s_mm, si - 1)  # r_ts freed by slot matmul
                rdst = r_ts[si % 2]
                wv = w_ts[n % 3][:, :KB * cnd].rearrange(
                    "p (r c) -> p r c", r=KB)
                gbc = (g_ts[n % 3][:, :cnd].unsqueeze(1)
                       .broadcast_to([128, KB, cnd]))
                v.tensor_mul(wv, gbc, wv)
                v.tensor_reduce(
                    out=rdst[:].rearrange("p (r b) -> p r b", r=KB)[
                        :, :, ch["xoff"]:ch["xoff"] + ch["nxg"]],
                    in_=w_ts[n % 3][:, :KB * cnd].rearrange(
                        "p (r x l) -> p r x l", r=KB, l=LD),
                    axis=mybir.AxisListType.X,
                    op=mybir.AluOpType.add,
                ).then_inc(s_v, 1)

        @block.tensor
        def _(t):
            t.wait_ge(s_in, 32)
            for si in range(SLOTS):
                t.wait_ge(s_v, slot_end[si])
                if si > 1:
                    t.wait_ge(s_cp, si - 1)  # psum freed by scalar copy
                t.matmul(psum_ts[si % 2][:], sel_t[:], r_ts[si % 2][:],
                         start=True, stop=True).then_inc(s_mm, 1)

        @block.scalar
        def _(sc):
            sc.dma_start(out=slab_t[:], in_=slab_d[:]).then_inc(s_in, 16)
            for si in range(SLOTS):
                sc.wait_ge(s_mm, si + 1)
                if si > 1:
                    sc.wait_ge(s_od, 16 * (si - 1))  # sino buf freed by DMA
                sc.copy(sino_ts[si % 2][:], psum_ts[si % 2][:]).then_inc(s_cp, 1)
                sc.dma_start(out=out_d[si], in_=sino_ts[si % 2][:]
                             ).then_inc(s_od, 16)
            sc.wait_ge(s_od, 16 * SLOTS)

    import concourse.mybir as mybir2
    mybir2.codegen_inst_isa_subclasses(nc)
    _PROG_CACHE["prog"] = nc
    return nc


def kernel(image):
    image = np.asarray(image, np.float32)
    assert image.shape == (BATCH, 1, IMG_SIZE, IMG_SIZE)
    plan = _get_plan()
    nc = _build_program(plan)

    from concourse.bass_utils import run_bass_kernel_spmd

    in_maps = []
    for ci, (var, angles) in enumerate(CORE_SPECS):
        in_maps.append({
            "slab": _build_slab(image, var),
            "idx": plan["core_idx"][ci],
            "w": plan["core_w"][ci],
            "sel": plan["sel"],
        })

    trace = bool(os.environ.get("RADON_TRACE"))
    if trace:
        _install_profhook()
    res = run_bass_kernel_spmd(nc, in_maps, list(range(8)), trace=trace)
    if trace:
        kernel.last_exec_time_ns = res.exec_time_ns

    sino = np.zeros((BATCH, 1, S, N_ANGLES), np.float32)
    for ci in range(8):
        o = res.results[ci]["out"]  # [SLOTS, 2, SXPAD]; slot 0 is dummy
        for si, k in enumerate(plan["core_order"][ci]):
            v = o[si + 1]  # [2, SXPAD]: col = r*NBUND + rankpos
            perm = plan["core_perm"][ci][si + 1]    # rankpos -> bundle
            full = v.reshape(2, KB, NBUND)          # [2, r, rankpos]
            binidx = perm[None, :] * KB + np.arange(KB)[:, None]  # [KB, rank]
            mask = binidx < S
            sino[:, 0, binidx[mask], k] = full[:, mask]
    return sino


def _install_profhook():
    import types
    if "antenv.axon_hooks" in sys.modules:
        return
    try:
        from trn_agent_boot.trn_boot import _ntff_profile_via_ctypes
        hook = _ntff_profile_via_ctypes("/opt/axon/libaxon_pjrt.so")
    except Exception:
        hook = None
    mod = types.ModuleType("antenv.axon_hooks")
    mod._hook = hook
    mod.set_axon_ntff_profile_hook = lambda h: setattr(mod, "_hook", h)
    mod.get_axon_ntff_profile_hook = lambda: mod._hook
    sys.modules["antenv.axon_hooks"] = mod
    import antenv
    antenv.axon_hooks = mod


if __name__ == "__main__":
    img = np.load("/tmp/ref_image.npy")
    out = kernel(image=img)
    exp = np.load("/tmp/ref_expected.npy")
    err = np.linalg.norm(out - exp) / np.linalg.norm(exp)
    print("kernel rel err:", err)
